# revision 37
# baseline (speedup 1.0000x reference)
# v6: head-parallel attention (2 heads/core over all 4096 tokens) — no K/V
# collective. Per core: project Q/K/V for its own heads from the full x^T
# (bf16 inputs, f32 accumulate), run attention software-pipelined so the PE
# never waits on the exp (PV for step k issues under the exp of step k+1),
# multiply by the core's Wo row slice, and combine partial attn_out with a
# ReduceScatter (4 pipelined 1024-token chunks overlapped with attention).
# Each core then owns 512 tokens (4 groups of 128): residual+LN1, FFN in
# bf16 split 384/128 tokens so FFN1 on the first three groups hides the last
# reduce-scatter, residual+LN2. DMAs are batched large and kept off the
# Activation queue during attention (its SEQ must keep dispatching exps).
import numpy as np

B, S, D = 2, 2048, 1024
H, DK, DVH, DFF = 16, 64, 64, 4096
N = B * S            # 4096 flattened tokens (b*S + s)
TOK = 512            # tokens owned per core after reduce-scatter
KC = D // 128        # 8
MH = DFF // 128      # 32
NG = 4               # reduce-scatter chunks (1024 tokens each)
EPS = 1e-5

_CACHE = {}


def _build():
    import concourse.mybir as mybir
    import concourse.tile as tile
    from concourse import bacc

    f32, f32r = mybir.dt.float32, mybir.dt.float32r
    bf16 = mybir.dt.bfloat16
    Exp = mybir.ActivationFunctionType.Exp
    Sqrt = mybir.ActivationFunctionType.Sqrt
    Ident = mybir.ActivationFunctionType.Identity
    AX = mybir.AxisListType.X
    Alu = mybir.AluOpType

    nc = bacc.Bacc("TRN2", target_bir_lowering=False, debug=False, num_devices=8)

    xT_d = nc.dram_tensor("xT", [D, N], bf16, kind="ExternalInput")
    xs_d = nc.dram_tensor("xs", [TOK, D], f32, kind="ExternalInput")
    wq_d = nc.dram_tensor("wq", [D, 128], bf16, kind="ExternalInput")
    wk_d = nc.dram_tensor("wk", [D, 128], bf16, kind="ExternalInput")
    wv_d = nc.dram_tensor("wv", [D, 128], bf16, kind="ExternalInput")
    wo_d = nc.dram_tensor("wo", [128, D], f32r, kind="ExternalInput")
    w1_d = nc.dram_tensor("w1", [D, DFF], bf16, kind="ExternalInput")
    w2_d = nc.dram_tensor("w2", [DFF, D], bf16, kind="ExternalInput")
    b1c_d = nc.dram_tensor("b1c", [128, MH], f32, kind="ExternalInput")
    b2r_d = nc.dram_tensor("b2r", [1, D], f32r, kind="ExternalInput")
    g1bc_d = nc.dram_tensor("g1bc", [128, D], f32, kind="ExternalInput")
    h1bc_d = nc.dram_tensor("h1bc", [128, D], f32, kind="ExternalInput")
    g2bc_d = nc.dram_tensor("g2bc", [128, D], f32, kind="ExternalInput")
    h2bc_d = nc.dram_tensor("h2bc", [128, D], f32, kind="ExternalInput")
    ident_d = nc.dram_tensor("ident", [128, 128], f32, kind="ExternalInput")
    identb_d = nc.dram_tensor("identb", [128, 128], bf16, kind="ExternalInput")
    ones64_d = nc.dram_tensor("ones64", [1, 64], f32r, kind="ExternalInput")
    ones128_d = nc.dram_tensor("ones128", [1, 128], f32r, kind="ExternalInput")
    onesv_d = nc.dram_tensor("onesv", [128, 64], f32r, kind="ExternalInput")
    y_d = nc.dram_tensor("y", [TOK, D], f32, kind="ExternalOutput")

    def ln_apply(pool, t, gbc, hbc, out_ap):
        # DVE-only layernorm: keeps the Activation engine free for the
        # attention exps (no act-table switches anywhere in the kernel).
        sums = pool.tile([128, 1], f32, tag="ln_sums", name="ln_sums")
        nc.vector.reduce_sum(sums[:], t[:], axis=AX)
        sq = pool.tile([128, D], f32, tag="ln_xa", name="ln_sq")
        ssq = pool.tile([128, 1], f32, tag="ln_ssq", name="ln_ssq")
        nc.scalar.activation(
            sq[:], t[:], mybir.ActivationFunctionType.Square, accum_out=ssq[:]
        )
        s2 = pool.tile([128, 1], f32, tag="ln_s2", name="ln_s2")
        nc.vector.tensor_mul(s2[:], sums[:], sums[:])
        var0 = pool.tile([128, 1], f32, tag="ln_var0", name="ln_var0")
        nc.vector.tensor_scalar(
            out=var0[:], in0=ssq[:], scalar1=1.0 / D, scalar2=EPS,
            op0=Alu.mult, op1=Alu.add,
        )
        s2b = pool.tile([128, 1], f32, tag="ln_s2b", name="ln_s2b")
        nc.vector.tensor_scalar_mul(s2b[:], s2[:], 1.0 / (D * D))
        var = pool.tile([128, 1], f32, tag="ln_var", name="ln_var")
        nc.vector.tensor_sub(var[:], var0[:], s2b[:])
        sd = pool.tile([128, 1], f32, tag="ln_sd", name="ln_sd")
        nc.scalar.activation(sd[:], var[:], Sqrt)
        rv = pool.tile([128, 1], f32, tag="ln_rv", name="ln_rv")
        nc.vector.reciprocal(rv[:], sd[:])
        nmr = pool.tile([128, 1], f32, tag="ln_nmr", name="ln_nmr")
        nc.vector.tensor_mul(nmr[:], sums[:], rv[:])
        nmr2 = pool.tile([128, 1], f32, tag="ln_nmr2", name="ln_nmr2")
        nc.vector.tensor_scalar_mul(nmr2[:], nmr[:], -1.0 / D)
        xa = pool.tile([128, D], f32, tag="ln_xa", name="ln_xa")
        nc.scalar.activation(xa[:], t[:], Ident, bias=nmr2[:], scale=rv[:])
        xg = pool.tile([128, D], f32, tag="ln_xg", name="ln_xg")
        nc.vector.tensor_mul(xg[:], xa[:], gbc[:])
        nc.vector.tensor_add(out_ap, xg[:], hbc[:])

    with tile.TileContext(nc) as tc:
        with (
            tc.tile_pool(name="const", bufs=1) as cpool,
            tc.tile_pool(name="lnp", bufs=1) as lnp,
            tc.tile_pool(name="rsp", bufs=2) as rsp,
            tc.tile_pool(name="rstp", bufs=3) as rstp,
            tc.tile_pool(name="w1pre", bufs=1) as w1pre_p,
            tc.tile_pool(name="dram", bufs=1, space="DRAM") as dram,
        ):
            x1 = cpool.tile([128, NG, D], bf16)
            x1T = cpool.tile([128, KC, TOK], bf16)

            rs_sizes = [2048, 1024, 1024]
            rs_in = [
                dram.tile([n, D], f32, name=f"rs_in{g}")
                for g, n in enumerate(rs_sizes)
            ]
            rs_out = [
                dram.tile([n // 8, D], f32, name=f"rs_out{g}")
                for g, n in enumerate(rs_sizes)
            ]
            # x1 group g reads rs_out[chunk][row-slice]
            rs_src = [(0, 0), (0, 128), (1, 0), (2, 0)]

            fence = cpool.tile([128, 1], f32)

            rst_eng = [nc.sync, nc.scalar, nc.gpsimd, nc.sync]

            def x1_ln(g, xsrow, g1, h1, gate=None):
                rst = rstp.tile([128, D], f32, tag="rst", name="rst")
                if gate is not None:
                    # tiny fenced write makes the DMA (a full-tile writer)
                    # wait for the attention fence, keeping its queue-blocking
                    # semaphore wait out of the attention-phase sync stream
                    nc.vector.tensor_copy(rst[0:1, 0:1], gate[0:1, :])
                ch, r0 = rs_src[g]
                rst_eng[g].dma_start(rst[:], rs_out[ch][r0:r0 + 128, :])
                t = rsp.tile([128, D], f32, tag="t1", name="t1")
                nc.vector.tensor_add(t[:], rst[:], xsrow)
                ln_apply(lnp, t, g1, h1, x1[:, g, :])

            w1pre = [
                w1pre_p.tile([128, KC, 512], bf16, name=f"w1pre{i}")
                for i in range(2)
            ]
            w2pre0 = w1pre_p.tile([128, 4, D], bf16, name="w2pre0")

            with tc.tile_pool(name="qkt", bufs=1) as qkt:
                qT = qkt.tile([128, N], f32r)
                kT = qkt.tile([128, N], f32r)
                v_sb = qkt.tile([128, N // 128, 2, 65], f32r)

                # gpsimd (SWDGE) carries all constants, ordered by first use
                wv_sb = qkt.tile([128, KC, 128], bf16)
                nc.gpsimd.dma_start(
                    wv_sb[:], wv_d.ap().rearrange("(kc p) m -> p kc m", p=128)
                )
                ident = cpool.tile([128, 128], f32)
                nc.gpsimd.dma_start(ident[:], ident_d.ap())
                nc.gpsimd.dma_start(
                    v_sb[:, :, :, 64:65].squeeze(3).rearrange("p a h -> p (a h)"),
                    onesv_d.ap(),
                )
                wo_sb = cpool.tile([128, D], f32r)
                nc.gpsimd.dma_start(wo_sb[:], wo_d.ap())
                ones64 = cpool.tile([1, 64], f32r)
                nc.gpsimd.dma_start(ones64[:], ones64_d.ap())
                ones128 = cpool.tile([1, 128], f32r)
                nc.gpsimd.dma_start(ones128[:], ones128_d.ap())
                identb = cpool.tile([128, 128], bf16)
                nc.gpsimd.dma_start(identb[:], identb_d.ap())
                b1c = cpool.tile([128, MH], f32)
                nc.gpsimd.dma_start(b1c[:], b1c_d.ap())
                b2r = cpool.tile([1, D], f32r)
                nc.gpsimd.dma_start(b2r[:], b2r_d.ap())
                g1bc = cpool.tile([128, D], f32)
                nc.gpsimd.dma_start(g1bc[:], g1bc_d.ap())
                h1bc = cpool.tile([128, D], f32)
                nc.gpsimd.dma_start(h1bc[:], h1bc_d.ap())
                xs = cpool.tile([128, NG, D], f32)
                nc.gpsimd.dma_start(
                    xs[:], xs_d.ap().rearrange("(g p) d -> p g d", p=128)
                )

                # ---- Phase 1: Q/K/V projections over all tokens
                with (
                    tc.tile_pool(name="wqk", bufs=1) as wqk,
                    tc.tile_pool(name="vtm", bufs=2) as vtm,
                    tc.tile_pool(name="xk", bufs=2) as xkp,
                    tc.tile_pool(name="ps_p", bufs=2, space="PSUM") as ps_p,
                    tc.tile_pool(name="ps_tr", bufs=2, space="PSUM") as ps_tr,
                ):
                    wq_sb = wqk.tile([128, KC, 128], bf16)
                    nc.sync.dma_start(
                        wq_sb[:], wq_d.ap().rearrange("(kc p) m -> p kc m", p=128)
                    )
                    wk_sb = wqk.tile([128, KC, 128], bf16)
                    nc.scalar.dma_start(
                        wk_sb[:], wk_d.ap().rearrange("(kc p) m -> p kc m", p=128)
                    )
                    for tg in range(8):
                        xk = xkp.tile([128, KC, 512], bf16, tag="xk",
                                      name=f"xk{tg}")
                        (nc.sync if tg % 2 == 0 else nc.scalar).dma_start(
                            xk[:],
                            xT_d.ap()[:, tg * 512:(tg + 1) * 512]
                            .rearrange("(kc p) t -> p kc t", p=128),
                        )
                        ps = ps_p.tile([128, 3, 512], f32, tag="psp", name="psp")
                        for kc in range(KC):
                            nc.tensor.matmul(
                                ps[:, 0, :], wq_sb[:, kc, :], xk[:, kc, :],
                                start=(kc == 0), stop=(kc == KC - 1),
                            )
                            nc.tensor.matmul(
                                ps[:, 1, :], wk_sb[:, kc, :], xk[:, kc, :],
                                start=(kc == 0), stop=(kc == KC - 1),
                            )
                            nc.tensor.matmul(
                                ps[:, 2, :], wv_sb[:, kc, :], xk[:, kc, :],
                                start=(kc == 0), stop=(kc == KC - 1),
                            )
                        nc.vector.tensor_copy(
                            qT[:, tg * 512:(tg + 1) * 512], ps[:, 0, :]
                        )
                        nc.vector.tensor_copy(
                            kT[:, tg * 512:(tg + 1) * 512], ps[:, 1, :]
                        )
                        vtmp = vtm.tile([128, 512], f32, tag="vtmp", name="vtmp")
                        nc.vector.tensor_copy(vtmp[:], ps[:, 2, :])
                        for tb in range(4):
                            pt = ps_tr.tile([128, 128], f32, tag="trp", name="trp")
                            nc.tensor.transpose(
                                pt[:], vtmp[:, tb * 128:(tb + 1) * 128], ident[:]
                            )
                            nc.vector.tensor_copy(
                                v_sb[:, tg * 4 + tb, :, 0:64],
                                pt[:].rearrange("p (h v) -> p h v", h=2),
                            )

                # ---- Phase 2: attention + Wo partials + pipelined RS
                with (
                    tc.tile_pool(name="at", bufs=3) as atpool,
                    tc.tile_pool(name="onq", bufs=2) as onqp,
                    tc.tile_pool(name="wosb", bufs=2) as wosp,
                    tc.tile_pool(name="rcp", bufs=1) as rcp,
                    tc.tile_pool(name="ps_s", bufs=2, space="PSUM") as ps_s,
                    tc.tile_pool(name="ps_o", bufs=2, space="PSUM") as ps_o,
                    tc.tile_pool(name="ps_r", bufs=1, space="PSUM") as ps_r,
                    tc.tile_pool(name="ps_w", bufs=1, space="PSUM") as ps_w,
                ):
                    for qc in range(8):
                        b = qc // 4
                        po = [
                            ps_o.tile([65, 512], f32, tag="po", name=f"po{qc}_{hh}")
                            for hh in range(2)
                        ]

                        def pv(prev):
                            pat, pg, phh = prev
                            for j in range(2):
                                kt = 2 * pg + j
                                nc.tensor.matmul(
                                    po[phh][:],
                                    v_sb[:, b * 16 + kt, phh, :],
                                    pat[:, j, :],
                                    start=(kt == 0), stop=(kt == 15),
                                )

                        prev = None
                        for g in range(8):
                            for hh in range(2):
                                sT = ps_s.tile([128, 2, 512], f32, tag="sT",
                                               name="sT")
                                for j in range(2):
                                    kt = 2 * g + j
                                    nc.tensor.matmul(
                                        sT[:, j, :],
                                        kT[hh * 64:(hh + 1) * 64,
                                           b * S + kt * 128:b * S + (kt + 1) * 128],
                                        qT[hh * 64:(hh + 1) * 64,
                                           qc * 512:(qc + 1) * 512],
                                        tile_position=(hh * 64, 0),
                                    )
                                at = atpool.tile([128, 2, 512], f32r, tag="at",
                                                 name="at")
                                nc.scalar.activation(
                                    at[:], sT[:], Exp, scale=0.125
                                )
                                if prev is not None:
                                    pv(prev)
                                prev = (at, g, hh)
                        pv(prev)

                        o_nq = onqp.tile([128, 512], f32r, tag="onq", name="onq")
                        for hh in range(2):
                            rec = rcp.tile([1, 512], f32r, tag="rec", name="rec")
                            with nc.allow_low_precision(reason="f32r"):
                                nc.vector.reciprocal(rec[:], po[hh][64:65, :])
                            rp = ps_r.tile([64, 512], f32, tag="rp", name="rp")
                            nc.tensor.matmul(rp[:], ones64[:], rec[:])
                            rsb = rcp.tile([64, 512], f32, tag="rsb", name="rsb")
                            nc.vector.tensor_copy(rsb[:], rp[:])
                            nc.vector.tensor_mul(
                                o_nq[hh * 64:(hh + 1) * 64, :],
                                po[hh][0:64, :],
                                rsb[:],
                            )
                        for half in range(2):
                            wos = wosp.tile([128, 2, D], f32, tag="wos",
                                            name="wos")
                            for sub in range(2):
                                tc4 = half * 2 + sub
                                for ncc in range(2):
                                    psw = ps_w.tile([128, 512], f32, tag="psw",
                                                    name="psw")
                                    nc.tensor.matmul(
                                        psw[:],
                                        o_nq[:, tc4 * 128:(tc4 + 1) * 128],
                                        wo_sb[:, ncc * 512:(ncc + 1) * 512],
                                    )
                                    nc.vector.tensor_copy(
                                        wos[:, sub, ncc * 512:(ncc + 1) * 512],
                                        psw[:],
                                    )
                            ch = 0 if qc < 4 else (1 if qc < 6 else 2)
                            qb = qc - (0, 4, 6)[ch]
                            r0 = qb * 512 + half * 256
                            nc.sync.dma_start(
                                rs_in[ch][r0:r0 + 256, :]
                                .rearrange("(a p) d -> p a d", p=128),
                                wos[:],
                            )
                        if qc == 7:
                            nc.vector.tensor_copy(fence[:], o_nq[:, 0:1])
                        if qc in (3, 5, 7):
                            ch = {3: 0, 5: 1, 7: 2}[qc]
                            nc.gpsimd.collective_compute(
                                "ReduceScatter",
                                Alu.add,
                                ins=[rs_in[ch].opt()],
                                outs=[rs_out[ch].opt()],
                                replica_groups=[[0, 1, 2, 3, 4, 5, 6, 7]],
                            )
                        if qc in (5, 6):
                            i = qc - 5
                            nc.gpsimd.dma_start(
                                w1pre[i][:],
                                w1_d.ap()[:, i * 512:(i + 1) * 512]
                                .rearrange("(kc p) m -> p kc m", p=128),
                            )
                        if qc == 4:
                            nc.gpsimd.dma_start(
                                w2pre0[:],
                                w2_d.ap()[0:512, :]
                                .rearrange("(a p) d -> p a d", p=128),
                            )

            # ---- Phase 3 + 4: LN1 (groups 0-2), x1 transpose, FFN

            with (
                tc.tile_pool(name="w1p", bufs=4) as w1p,
                tc.tile_pool(name="htp", bufs=1) as htp,
                tc.tile_pool(name="w2p", bufs=3) as w2p,
                tc.tile_pool(name="ffc", bufs=1) as ffc,
                tc.tile_pool(name="outp", bufs=1) as outp,
            ):
                g2bc = ffc.tile([128, D], f32)
                nc.gpsimd.dma_start(g2bc[:], g2bc_d.ap())
                h2bc = ffc.tile([128, D], f32)
                nc.gpsimd.dma_start(h2bc[:], h2bc_d.ap())
                hT = htp.tile([128, MH, TOK], bf16)
                w1tiles = {}
                w2tiles = {0: w2pre0}

                def f2_finish(mt, psy_pair):
                    for ncc in range(2):
                        nc.tensor.matmul(
                            psy_pair[ncc][:],
                            ones128[:],
                            b2r[:, ncc * 512:(ncc + 1) * 512],
                            start=False, stop=True,
                        )
                    t2 = outp.tile([128, D], f32, tag="t2", name="t2")
                    for ncc in range(2):
                        nc.vector.tensor_add(
                            t2[:, ncc * 512:(ncc + 1) * 512],
                            psy_pair[ncc][:],
                            x1[:, mt, ncc * 512:(ncc + 1) * 512],
                        )
                    ot = outp.tile([128, D], f32, tag="ot", name="ot")
                    ln_apply(lnp, t2, g2bc, h2bc, ot[:])
                    nc.sync.dma_start(
                        y_d.ap()[mt * 128:(mt + 1) * 128, :], ot[:]
                    )

                def f1b(bg, ps_f1b):
                    bt = w1tiles.pop(bg)
                    for a in range(4):
                        mh = 4 * bg + a
                        psb = ps_f1b.tile([128, 128], f32, tag="psf1b",
                                          name="psf1b")
                        for dc in range(KC):
                            nc.tensor.matmul(
                                psb[:], bt[:, dc, a * 128:(a + 1) * 128],
                                x1T[:, dc, 384:512],
                                start=(dc == 0), stop=(dc == KC - 1),
                            )
                        nc.vector.tensor_scalar(
                            out=hT[:, mh, 384:512], in0=psb[:],
                            scalar1=b1c[:, mh:mh + 1], scalar2=0.0,
                            op0=Alu.add, op1=Alu.max,
                        )

                with (
                    tc.tile_pool(name="ps_t2", bufs=1, space="PSUM") as ps_t2,
                    tc.tile_pool(name="ps_f1a", bufs=2, space="PSUM") as ps_f1a,
                    tc.tile_pool(name="ps_f1b", bufs=1, space="PSUM") as ps_f1b,
                    tc.tile_pool(name="ps_f2a", bufs=1, space="PSUM") as ps_f2a,
                ):
                    psyA = [
                        [
                            ps_f2a.tile([128, 512], f32, tag=f"pyA{mt}{ncc}",
                                        name=f"pyA{mt}{ncc}")
                            for ncc in range(2)
                        ]
                        for mt in range(2)
                    ]

                    def f2a(bg, w2t):
                        # FFN2 accumulation for token groups 0-1, fused into
                        # the FFN1 stream as each mh block of hT completes
                        for a in range(4):
                            mh = 4 * bg + a
                            for mt in range(2):
                                for ncc in range(2):
                                    nc.tensor.matmul(
                                        psyA[mt][ncc][:],
                                        hT[:, mh, mt * 128:(mt + 1) * 128],
                                        w2t[:, a, ncc * 512:(ncc + 1) * 512],
                                        start=(mh == 0), stop=False,
                                    )

                    def x1_transpose(g):
                        for dc in range(KC):
                            pt2 = ps_t2.tile([128, 128], bf16, tag="trp2",
                                             name="trp2")
                            nc.tensor.transpose(
                                pt2[:], x1[:, g, dc * 128:(dc + 1) * 128],
                                identb[:],
                            )
                            nc.vector.tensor_copy(
                                x1T[:, dc, g * 128:(g + 1) * 128], pt2[:]
                            )

                    for g in range(3):
                        x1_ln(g, xs[:, g, :], g1bc, h1bc, gate=fence[:])
                        x1_transpose(g)

                    def bstep(bg):
                        f1b(bg, ps_f1b)
                        f2a(bg, w2tiles.pop(bg))
                        if bg + 1 < 8:
                            w2t = w2p.tile([128, 4, D], bf16, tag="w2t",
                                           name="w2t")
                            (nc.sync if bg % 2 == 0 else nc.scalar).dma_start(
                                w2t[:],
                                w2_d.ap()[(bg + 1) * 512:(bg + 2) * 512, :]
                                .rearrange("(a p) d -> p a d", p=128),
                            )
                            w2tiles[bg + 1] = w2t

                    for g4 in range(8):
                        if g4 < 2:
                            w1t = w1pre[g4]
                        else:
                            w1t = w1p.tile([128, KC, 512], bf16, tag="w1t",
                                           name="w1t")
                            (nc.sync if g4 % 2 == 0 else nc.scalar).dma_start(
                                w1t[:],
                                w1_d.ap()[:, g4 * 512:(g4 + 1) * 512]
                                .rearrange("(kc p) m -> p kc m", p=128),
                            )
                        w1tiles[g4] = w1t
                        for a in range(4):
                            mh = 4 * g4 + a
                            psa = ps_f1a.tile([128, 512], f32, tag="psf1a",
                                              name="psf1a")
                            for dc in range(KC):
                                nc.tensor.matmul(
                                    psa[:, 0:384],
                                    w1t[:, dc, a * 128:(a + 1) * 128],
                                    x1T[:, dc, 0:384],
                                    start=(dc == 0), stop=(dc == KC - 1),
                                )
                            nc.vector.tensor_scalar(
                                out=hT[:, mh, 0:384], in0=psa[:, 0:384],
                                scalar1=b1c[:, mh:mh + 1], scalar2=0.0,
                                op0=Alu.add, op1=Alu.max,
                            )
                        if g4 == 3:
                            x1_ln(3, xs[:, 3, :], g1bc, h1bc)
                            x1_transpose(3)
                        if g4 >= 4:
                            bstep(g4 - 4)
                    for bg in range(4, 8):
                        bstep(bg)
                    f2_finish(0, psyA[0])
                    f2_finish(1, psyA[1])

                # ---- FFN2 second half (token groups 2-3) + residual + LN2
                with tc.tile_pool(name="ps_f2b", bufs=1, space="PSUM") as ps_f2b:
                    psyB = [
                        [
                            ps_f2b.tile([128, 512], f32, tag=f"pyB{mt}{ncc}",
                                        name=f"pyB{mt}{ncc}")
                            for ncc in range(2)
                        ]
                        for mt in range(2)
                    ]
                    for wc in range(8):
                        w2t = w2p.tile([128, 4, D], bf16, tag="w2t", name="w2t")
                        (nc.sync if wc % 2 == 0 else nc.scalar).dma_start(
                            w2t[:],
                            w2_d.ap()[wc * 512:(wc + 1) * 512, :]
                            .rearrange("(a p) d -> p a d", p=128),
                        )
                        for a in range(4):
                            mh = 4 * wc + a
                            for mt in (2, 3):
                                for ncc in range(2):
                                    nc.tensor.matmul(
                                        psyB[mt - 2][ncc][:],
                                        hT[:, mh, mt * 128:(mt + 1) * 128],
                                        w2t[:, a, ncc * 512:(ncc + 1) * 512],
                                        start=(mh == 0), stop=False,
                                    )
                    f2_finish(2, psyB[0])
                    f2_finish(3, psyB[1])
    nc.compile()
    return nc


def _core_rows(c):
    # 3 reduce-scatter chunks: tokens 0-2047 scattered in 256-row blocks,
    # tokens 2048-3071 and 3072-4095 in 128-row blocks.
    return np.concatenate([
        c * 256 + np.arange(256),
        2048 + c * 128 + np.arange(128),
        3072 + c * 128 + np.arange(128),
    ])


def _in_maps(x, Wq, Wk, Wv, Wo, ln1_g, ln1_b, W1, b1, W2, b2, ln2_g, ln2_b):
    import ml_dtypes

    bf16 = ml_dtypes.bfloat16
    xf = np.ascontiguousarray(np.asarray(x, np.float32).reshape(N, D))
    xT = np.ascontiguousarray(xf.T.astype(bf16))
    Wq = np.asarray(Wq, np.float32)
    Wk = np.asarray(Wk, np.float32)
    Wv = np.asarray(Wv, np.float32)
    Wo = np.asarray(Wo, np.float32)
    bcast = lambda v: np.ascontiguousarray(
        np.broadcast_to(np.asarray(v, np.float32), (128, D))
    )
    common = {
        "xT": xT,
        "w1": np.ascontiguousarray(np.asarray(W1, np.float32).astype(bf16)),
        "w2": np.ascontiguousarray(np.asarray(W2, np.float32).astype(bf16)),
        "b1c": np.ascontiguousarray(np.asarray(b1, np.float32).reshape(MH, 128).T),
        "b2r": np.ascontiguousarray(np.asarray(b2, np.float32).reshape(1, D)),
        "g1bc": bcast(ln1_g), "h1bc": bcast(ln1_b),
        "g2bc": bcast(ln2_g), "h2bc": bcast(ln2_b),
        "ident": np.eye(128, dtype=np.float32),
        "identb": np.eye(128, dtype=np.float32).astype(bf16),
        "ones64": np.ones((1, 64), np.float32),
        "ones128": np.ones((1, 128), np.float32),
        "onesv": np.ones((128, 64), np.float32),
    }
    in_maps = []
    for c in range(8):
        h0 = 2 * c
        m = dict(common)
        m["wq"] = np.ascontiguousarray(
            Wq[h0:h0 + 2].transpose(1, 0, 2).reshape(D, 128).astype(bf16)
        )
        m["wk"] = np.ascontiguousarray(
            Wk[h0:h0 + 2].transpose(1, 0, 2).reshape(D, 128).astype(bf16)
        )
        m["wv"] = np.ascontiguousarray(
            Wv[h0:h0 + 2].transpose(1, 0, 2).reshape(D, 128).astype(bf16)
        )
        m["wo"] = np.ascontiguousarray(Wo[h0 * 64:h0 * 64 + 128, :])
        m["xs"] = np.ascontiguousarray(xf[_core_rows(c)])
        in_maps.append(m)
    return in_maps


def kernel(x, Wq, Wk, Wv, Wo, ln1_g, ln1_b, W1, b1, W2, b2, ln2_g, ln2_b):
    from concourse.bass_utils import run_bass_kernel_spmd

    if "nc" not in _CACHE:
        _CACHE["nc"] = _build()
    nc = _CACHE["nc"]
    in_maps = _in_maps(x, Wq, Wk, Wv, Wo, ln1_g, ln1_b, W1, b1, W2, b2, ln2_g, ln2_b)
    res = run_bass_kernel_spmd(nc, in_maps, core_ids=list(range(8)))
    out = np.empty((N, D), np.float32)
    for c in range(8):
        out[_core_rows(c)] = res.results[c]["y"]
    return out.reshape(B, S, D)


# revision 38
# speedup vs baseline: 1.0118x; 1.0118x over previous
# v6: head-parallel attention (2 heads/core over all 4096 tokens) — no K/V
# collective. Per core: project Q/K/V for its own heads from the full x^T
# (bf16 inputs, f32 accumulate), run attention software-pipelined so the PE
# never waits on the exp (PV for step k issues under the exp of step k+1),
# multiply by the core's Wo row slice, and combine partial attn_out with a
# ReduceScatter (4 pipelined 1024-token chunks overlapped with attention).
# Each core then owns 512 tokens (4 groups of 128): residual+LN1, FFN in
# bf16 split 384/128 tokens so FFN1 on the first three groups hides the last
# reduce-scatter, residual+LN2. DMAs are batched large and kept off the
# Activation queue during attention (its SEQ must keep dispatching exps).
import numpy as np

B, S, D = 2, 2048, 1024
H, DK, DVH, DFF = 16, 64, 64, 4096
N = B * S            # 4096 flattened tokens (b*S + s)
TOK = 512            # tokens owned per core after reduce-scatter
KC = D // 128        # 8
MH = DFF // 128      # 32
NG = 4               # reduce-scatter chunks (1024 tokens each)
EPS = 1e-5

_CACHE = {}


def _build():
    import concourse.mybir as mybir
    import concourse.tile as tile
    from concourse import bacc

    f32, f32r = mybir.dt.float32, mybir.dt.float32r
    bf16 = mybir.dt.bfloat16
    Exp = mybir.ActivationFunctionType.Exp
    Sqrt = mybir.ActivationFunctionType.Sqrt
    Ident = mybir.ActivationFunctionType.Identity
    AX = mybir.AxisListType.X
    Alu = mybir.AluOpType

    nc = bacc.Bacc("TRN2", target_bir_lowering=False, debug=False, num_devices=8)

    xT_d = nc.dram_tensor("xT", [D, N], bf16, kind="ExternalInput")
    xs_d = nc.dram_tensor("xs", [TOK, D], f32, kind="ExternalInput")
    wq_d = nc.dram_tensor("wq", [D, 128], bf16, kind="ExternalInput")
    wk_d = nc.dram_tensor("wk", [D, 128], bf16, kind="ExternalInput")
    wv_d = nc.dram_tensor("wv", [D, 128], bf16, kind="ExternalInput")
    wo_d = nc.dram_tensor("wo", [128, D], f32r, kind="ExternalInput")
    w1_d = nc.dram_tensor("w1", [D, DFF], bf16, kind="ExternalInput")
    w2_d = nc.dram_tensor("w2", [DFF, D], bf16, kind="ExternalInput")
    b1c_d = nc.dram_tensor("b1c", [128, MH], f32, kind="ExternalInput")
    b2r_d = nc.dram_tensor("b2r", [1, D], f32r, kind="ExternalInput")
    g1bc_d = nc.dram_tensor("g1bc", [128, D], f32, kind="ExternalInput")
    h1bc_d = nc.dram_tensor("h1bc", [128, D], f32, kind="ExternalInput")
    g2bc_d = nc.dram_tensor("g2bc", [128, D], f32, kind="ExternalInput")
    h2bc_d = nc.dram_tensor("h2bc", [128, D], f32, kind="ExternalInput")
    ident_d = nc.dram_tensor("ident", [128, 128], f32, kind="ExternalInput")
    identb_d = nc.dram_tensor("identb", [128, 128], bf16, kind="ExternalInput")
    ones64_d = nc.dram_tensor("ones64", [1, 64], f32r, kind="ExternalInput")
    ones128_d = nc.dram_tensor("ones128", [1, 128], f32r, kind="ExternalInput")
    onesv_d = nc.dram_tensor("onesv", [128, 64], f32r, kind="ExternalInput")
    y_d = nc.dram_tensor("y", [TOK, D], f32, kind="ExternalOutput")

    def ln_apply(pool, t, gbc, hbc, out_ap):
        # DVE-only layernorm: keeps the Activation engine free for the
        # attention exps (no act-table switches anywhere in the kernel).
        sums = pool.tile([128, 1], f32, tag="ln_sums", name="ln_sums")
        nc.vector.reduce_sum(sums[:], t[:], axis=AX)
        sq = pool.tile([128, D], f32, tag="ln_xa", name="ln_sq")
        ssq = pool.tile([128, 1], f32, tag="ln_ssq", name="ln_ssq")
        nc.scalar.activation(
            sq[:], t[:], mybir.ActivationFunctionType.Square, accum_out=ssq[:]
        )
        s2 = pool.tile([128, 1], f32, tag="ln_s2", name="ln_s2")
        nc.vector.tensor_mul(s2[:], sums[:], sums[:])
        var0 = pool.tile([128, 1], f32, tag="ln_var0", name="ln_var0")
        nc.vector.tensor_scalar(
            out=var0[:], in0=ssq[:], scalar1=1.0 / D, scalar2=EPS,
            op0=Alu.mult, op1=Alu.add,
        )
        s2b = pool.tile([128, 1], f32, tag="ln_s2b", name="ln_s2b")
        nc.vector.tensor_scalar_mul(s2b[:], s2[:], 1.0 / (D * D))
        var = pool.tile([128, 1], f32, tag="ln_var", name="ln_var")
        nc.vector.tensor_sub(var[:], var0[:], s2b[:])
        sd = pool.tile([128, 1], f32, tag="ln_sd", name="ln_sd")
        nc.scalar.activation(sd[:], var[:], Sqrt)
        rv = pool.tile([128, 1], f32, tag="ln_rv", name="ln_rv")
        nc.vector.reciprocal(rv[:], sd[:])
        nmr = pool.tile([128, 1], f32, tag="ln_nmr", name="ln_nmr")
        nc.vector.tensor_mul(nmr[:], sums[:], rv[:])
        nmr2 = pool.tile([128, 1], f32, tag="ln_nmr2", name="ln_nmr2")
        nc.vector.tensor_scalar_mul(nmr2[:], nmr[:], -1.0 / D)
        xa = pool.tile([128, D], f32, tag="ln_xa", name="ln_xa")
        nc.scalar.activation(xa[:], t[:], Ident, bias=nmr2[:], scale=rv[:])
        xg = pool.tile([128, D], f32, tag="ln_xg", name="ln_xg")
        nc.vector.tensor_mul(xg[:], xa[:], gbc[:])
        nc.vector.tensor_add(out_ap, xg[:], hbc[:])

    with tile.TileContext(nc) as tc:
        with (
            tc.tile_pool(name="const", bufs=1) as cpool,
            tc.tile_pool(name="lnp", bufs=2) as lnp,
            tc.tile_pool(name="rsp", bufs=2) as rsp,
            tc.tile_pool(name="rstp", bufs=3) as rstp,
            tc.tile_pool(name="w1pre", bufs=1) as w1pre_p,
            tc.tile_pool(name="dram", bufs=1, space="DRAM") as dram,
        ):
            x1 = cpool.tile([128, NG, D], bf16)
            x1T = cpool.tile([128, KC, TOK], bf16)

            rs_sizes = [2048, 1024, 1024]
            rs_in = [
                dram.tile([n, D], f32, name=f"rs_in{g}")
                for g, n in enumerate(rs_sizes)
            ]
            rs_out = [
                dram.tile([n // 8, D], f32, name=f"rs_out{g}")
                for g, n in enumerate(rs_sizes)
            ]
            # x1 group g reads rs_out[chunk][row-slice]
            rs_src = [(0, 0), (0, 128), (1, 0), (2, 0)]

            fence = cpool.tile([128, 1], f32)

            rst_eng = [nc.sync, nc.scalar, nc.gpsimd, nc.sync]

            def x1_ln(g, xsrow, g1, h1, gate=None):
                rst = rstp.tile([128, D], f32, tag="rst", name="rst")
                if gate is not None:
                    # tiny fenced write makes the DMA (a full-tile writer)
                    # wait for the attention fence, keeping its queue-blocking
                    # semaphore wait out of the attention-phase sync stream
                    nc.vector.tensor_copy(rst[0:1, 0:1], gate[0:1, :])
                ch, r0 = rs_src[g]
                rst_eng[g].dma_start(rst[:], rs_out[ch][r0:r0 + 128, :])
                t = rsp.tile([128, D], f32, tag="t1", name="t1")
                nc.vector.tensor_add(t[:], rst[:], xsrow)
                ln_apply(lnp, t, g1, h1, x1[:, g, :])

            w1pre = [
                w1pre_p.tile([128, KC, 512], bf16, name=f"w1pre{i}")
                for i in range(2)
            ]
            w2pre0 = w1pre_p.tile([128, 4, D], bf16, name="w2pre0")

            with tc.tile_pool(name="qkt", bufs=1) as qkt:
                qT = qkt.tile([128, N], f32r)
                kT = qkt.tile([128, N], f32r)
                v_sb = qkt.tile([128, N // 128, 2, 65], f32r)

                # gpsimd (SWDGE) carries all constants, ordered by first use
                wv_sb = qkt.tile([128, KC, 128], bf16)
                nc.gpsimd.dma_start(
                    wv_sb[:], wv_d.ap().rearrange("(kc p) m -> p kc m", p=128)
                )
                ident = cpool.tile([128, 128], f32)
                nc.gpsimd.dma_start(ident[:], ident_d.ap())
                nc.gpsimd.dma_start(
                    v_sb[:, :, :, 64:65].squeeze(3).rearrange("p a h -> p (a h)"),
                    onesv_d.ap(),
                )
                wo_sb = cpool.tile([128, D], f32r)
                nc.gpsimd.dma_start(wo_sb[:], wo_d.ap())
                ones64 = cpool.tile([1, 64], f32r)
                nc.gpsimd.dma_start(ones64[:], ones64_d.ap())
                ones128 = cpool.tile([1, 128], f32r)
                nc.gpsimd.dma_start(ones128[:], ones128_d.ap())
                identb = cpool.tile([128, 128], bf16)
                nc.gpsimd.dma_start(identb[:], identb_d.ap())
                b1c = cpool.tile([128, MH], f32)
                nc.gpsimd.dma_start(b1c[:], b1c_d.ap())
                b2r = cpool.tile([1, D], f32r)
                nc.gpsimd.dma_start(b2r[:], b2r_d.ap())
                g1bc = cpool.tile([128, D], f32)
                nc.gpsimd.dma_start(g1bc[:], g1bc_d.ap())
                h1bc = cpool.tile([128, D], f32)
                nc.gpsimd.dma_start(h1bc[:], h1bc_d.ap())
                xs = cpool.tile([128, NG, D], f32)
                nc.gpsimd.dma_start(
                    xs[:], xs_d.ap().rearrange("(g p) d -> p g d", p=128)
                )

                # ---- Phase 1: Q/K/V projections over all tokens
                with (
                    tc.tile_pool(name="wqk", bufs=1) as wqk,
                    tc.tile_pool(name="vtm", bufs=2) as vtm,
                    tc.tile_pool(name="xk", bufs=2) as xkp,
                    tc.tile_pool(name="ps_p", bufs=2, space="PSUM") as ps_p,
                    tc.tile_pool(name="ps_tr", bufs=2, space="PSUM") as ps_tr,
                ):
                    wq_sb = wqk.tile([128, KC, 128], bf16)
                    nc.sync.dma_start(
                        wq_sb[:], wq_d.ap().rearrange("(kc p) m -> p kc m", p=128)
                    )
                    wk_sb = wqk.tile([128, KC, 128], bf16)
                    nc.scalar.dma_start(
                        wk_sb[:], wk_d.ap().rearrange("(kc p) m -> p kc m", p=128)
                    )
                    for tg in range(8):
                        xk = xkp.tile([128, KC, 512], bf16, tag="xk",
                                      name=f"xk{tg}")
                        (nc.sync if tg % 2 == 0 else nc.scalar).dma_start(
                            xk[:],
                            xT_d.ap()[:, tg * 512:(tg + 1) * 512]
                            .rearrange("(kc p) t -> p kc t", p=128),
                        )
                        ps = ps_p.tile([128, 3, 512], f32, tag="psp", name="psp")
                        for kc in range(KC):
                            nc.tensor.matmul(
                                ps[:, 0, :], wq_sb[:, kc, :], xk[:, kc, :],
                                start=(kc == 0), stop=(kc == KC - 1),
                            )
                            nc.tensor.matmul(
                                ps[:, 1, :], wk_sb[:, kc, :], xk[:, kc, :],
                                start=(kc == 0), stop=(kc == KC - 1),
                            )
                            nc.tensor.matmul(
                                ps[:, 2, :], wv_sb[:, kc, :], xk[:, kc, :],
                                start=(kc == 0), stop=(kc == KC - 1),
                            )
                        nc.vector.tensor_copy(
                            qT[:, tg * 512:(tg + 1) * 512], ps[:, 0, :]
                        )
                        nc.vector.tensor_copy(
                            kT[:, tg * 512:(tg + 1) * 512], ps[:, 1, :]
                        )
                        vtmp = vtm.tile([128, 512], f32, tag="vtmp", name="vtmp")
                        nc.vector.tensor_copy(vtmp[:], ps[:, 2, :])
                        for tb in range(4):
                            pt = ps_tr.tile([128, 128], f32, tag="trp", name="trp")
                            nc.tensor.transpose(
                                pt[:], vtmp[:, tb * 128:(tb + 1) * 128], ident[:]
                            )
                            nc.vector.tensor_copy(
                                v_sb[:, tg * 4 + tb, :, 0:64],
                                pt[:].rearrange("p (h v) -> p h v", h=2),
                            )

                # ---- Phase 2: attention + Wo partials + pipelined RS
                with (
                    tc.tile_pool(name="at", bufs=3) as atpool,
                    tc.tile_pool(name="onq", bufs=2) as onqp,
                    tc.tile_pool(name="wosb", bufs=2) as wosp,
                    tc.tile_pool(name="rcp", bufs=1) as rcp,
                    tc.tile_pool(name="ps_s", bufs=2, space="PSUM") as ps_s,
                    tc.tile_pool(name="ps_o", bufs=2, space="PSUM") as ps_o,
                    tc.tile_pool(name="ps_r", bufs=1, space="PSUM") as ps_r,
                    tc.tile_pool(name="ps_w", bufs=1, space="PSUM") as ps_w,
                ):
                    for qc in range(8):
                        b = qc // 4
                        po = [
                            ps_o.tile([65, 512], f32, tag="po", name=f"po{qc}_{hh}")
                            for hh in range(2)
                        ]

                        def pv(prev):
                            pat, pg, phh = prev
                            for j in range(2):
                                kt = 2 * pg + j
                                nc.tensor.matmul(
                                    po[phh][:],
                                    v_sb[:, b * 16 + kt, phh, :],
                                    pat[:, j, :],
                                    start=(kt == 0), stop=(kt == 15),
                                )

                        prev = None
                        for g in range(8):
                            for hh in range(2):
                                sT = ps_s.tile([128, 2, 512], f32, tag="sT",
                                               name="sT")
                                for j in range(2):
                                    kt = 2 * g + j
                                    nc.tensor.matmul(
                                        sT[:, j, :],
                                        kT[hh * 64:(hh + 1) * 64,
                                           b * S + kt * 128:b * S + (kt + 1) * 128],
                                        qT[hh * 64:(hh + 1) * 64,
                                           qc * 512:(qc + 1) * 512],
                                        tile_position=(hh * 64, 0),
                                    )
                                at = atpool.tile([128, 2, 512], f32r, tag="at",
                                                 name="at")
                                nc.scalar.activation(
                                    at[:], sT[:], Exp, scale=0.125
                                )
                                if prev is not None:
                                    pv(prev)
                                prev = (at, g, hh)
                        pv(prev)

                        o_nq = onqp.tile([128, 512], f32r, tag="onq", name="onq")
                        for hh in range(2):
                            rec = rcp.tile([1, 512], f32r, tag="rec", name="rec")
                            with nc.allow_low_precision(reason="f32r"):
                                nc.vector.reciprocal(rec[:], po[hh][64:65, :])
                            rp = ps_r.tile([64, 512], f32, tag="rp", name="rp")
                            nc.tensor.matmul(rp[:], ones64[:], rec[:])
                            rsb = rcp.tile([64, 512], f32, tag="rsb", name="rsb")
                            nc.vector.tensor_copy(rsb[:], rp[:])
                            nc.vector.tensor_mul(
                                o_nq[hh * 64:(hh + 1) * 64, :],
                                po[hh][0:64, :],
                                rsb[:],
                            )
                        for half in range(2):
                            wos = wosp.tile([128, 2, D], f32, tag="wos",
                                            name="wos")
                            for sub in range(2):
                                tc4 = half * 2 + sub
                                for ncc in range(2):
                                    psw = ps_w.tile([128, 512], f32, tag="psw",
                                                    name="psw")
                                    nc.tensor.matmul(
                                        psw[:],
                                        o_nq[:, tc4 * 128:(tc4 + 1) * 128],
                                        wo_sb[:, ncc * 512:(ncc + 1) * 512],
                                    )
                                    nc.vector.tensor_copy(
                                        wos[:, sub, ncc * 512:(ncc + 1) * 512],
                                        psw[:],
                                    )
                            ch = 0 if qc < 4 else (1 if qc < 6 else 2)
                            qb = qc - (0, 4, 6)[ch]
                            r0 = qb * 512 + half * 256
                            nc.sync.dma_start(
                                rs_in[ch][r0:r0 + 256, :]
                                .rearrange("(a p) d -> p a d", p=128),
                                wos[:],
                            )
                        if qc == 7:
                            nc.vector.tensor_copy(fence[:], o_nq[:, 0:1])
                        if qc in (3, 5, 7):
                            ch = {3: 0, 5: 1, 7: 2}[qc]
                            nc.gpsimd.collective_compute(
                                "ReduceScatter",
                                Alu.add,
                                ins=[rs_in[ch].opt()],
                                outs=[rs_out[ch].opt()],
                                replica_groups=[[0, 1, 2, 3, 4, 5, 6, 7]],
                            )
                        if qc in (5, 6):
                            i = qc - 5
                            nc.gpsimd.dma_start(
                                w1pre[i][:],
                                w1_d.ap()[:, i * 512:(i + 1) * 512]
                                .rearrange("(kc p) m -> p kc m", p=128),
                            )
                        if qc == 4:
                            nc.gpsimd.dma_start(
                                w2pre0[:],
                                w2_d.ap()[0:512, :]
                                .rearrange("(a p) d -> p a d", p=128),
                            )

            # ---- Phase 3 + 4: LN1 (groups 0-2), x1 transpose, FFN

            with (
                tc.tile_pool(name="w1p", bufs=3) as w1p,
                tc.tile_pool(name="htp", bufs=1) as htp,
                tc.tile_pool(name="w2p", bufs=3) as w2p,
                tc.tile_pool(name="ffc", bufs=1) as ffc,
                tc.tile_pool(name="outp", bufs=1) as outp,
            ):
                g2bc = ffc.tile([128, D], f32)
                nc.gpsimd.dma_start(g2bc[:], g2bc_d.ap())
                h2bc = ffc.tile([128, D], f32)
                nc.gpsimd.dma_start(h2bc[:], h2bc_d.ap())
                hT = htp.tile([128, MH, TOK], bf16)
                w1tiles = {}
                w2tiles = {0: w2pre0}

                def f2_finish(mt, psy_pair):
                    for ncc in range(2):
                        nc.tensor.matmul(
                            psy_pair[ncc][:],
                            ones128[:],
                            b2r[:, ncc * 512:(ncc + 1) * 512],
                            start=False, stop=True,
                        )
                    t2 = outp.tile([128, D], f32, tag="t2", name="t2")
                    for ncc in range(2):
                        nc.vector.tensor_add(
                            t2[:, ncc * 512:(ncc + 1) * 512],
                            psy_pair[ncc][:],
                            x1[:, mt, ncc * 512:(ncc + 1) * 512],
                        )
                    ot = outp.tile([128, D], f32, tag="ot", name="ot")
                    ln_apply(lnp, t2, g2bc, h2bc, ot[:])
                    nc.sync.dma_start(
                        y_d.ap()[mt * 128:(mt + 1) * 128, :], ot[:]
                    )

                def f1b(bg, ps_f1b):
                    bt = w1tiles.pop(bg)
                    for a in range(4):
                        mh = 4 * bg + a
                        psb = ps_f1b.tile([128, 128], f32, tag="psf1b",
                                          name="psf1b")
                        for dc in range(KC):
                            nc.tensor.matmul(
                                psb[:], bt[:, dc, a * 128:(a + 1) * 128],
                                x1T[:, dc, 384:512],
                                start=(dc == 0), stop=(dc == KC - 1),
                            )
                        nc.vector.tensor_scalar(
                            out=hT[:, mh, 384:512], in0=psb[:],
                            scalar1=b1c[:, mh:mh + 1], scalar2=0.0,
                            op0=Alu.add, op1=Alu.max,
                        )

                with (
                    tc.tile_pool(name="ps_t2", bufs=1, space="PSUM") as ps_t2,
                    tc.tile_pool(name="ps_f1a", bufs=2, space="PSUM") as ps_f1a,
                    tc.tile_pool(name="ps_f1b", bufs=1, space="PSUM") as ps_f1b,
                    tc.tile_pool(name="ps_f2a", bufs=1, space="PSUM") as ps_f2a,
                ):
                    psyA = [
                        [
                            ps_f2a.tile([128, 512], f32, tag=f"pyA{mt}{ncc}",
                                        name=f"pyA{mt}{ncc}")
                            for ncc in range(2)
                        ]
                        for mt in range(2)
                    ]

                    def f2a(bg, w2t):
                        # FFN2 accumulation for token groups 0-1, fused into
                        # the FFN1 stream as each mh block of hT completes
                        for a in range(4):
                            mh = 4 * bg + a
                            for mt in range(2):
                                for ncc in range(2):
                                    nc.tensor.matmul(
                                        psyA[mt][ncc][:],
                                        hT[:, mh, mt * 128:(mt + 1) * 128],
                                        w2t[:, a, ncc * 512:(ncc + 1) * 512],
                                        start=(mh == 0), stop=False,
                                    )

                    def x1_transpose(g):
                        for dc in range(KC):
                            pt2 = ps_t2.tile([128, 128], bf16, tag="trp2",
                                             name="trp2")
                            nc.tensor.transpose(
                                pt2[:], x1[:, g, dc * 128:(dc + 1) * 128],
                                identb[:],
                            )
                            nc.vector.tensor_copy(
                                x1T[:, dc, g * 128:(g + 1) * 128], pt2[:]
                            )

                    for g in range(3):
                        x1_ln(g, xs[:, g, :], g1bc, h1bc, gate=fence[:])
                        x1_transpose(g)

                    def bstep(bg):
                        f1b(bg, ps_f1b)
                        f2a(bg, w2tiles.pop(bg))
                        if bg + 1 < 8:
                            w2t = w2p.tile([128, 4, D], bf16, tag="w2t",
                                           name="w2t")
                            (nc.sync if bg % 2 == 0 else nc.scalar).dma_start(
                                w2t[:],
                                w2_d.ap()[(bg + 1) * 512:(bg + 2) * 512, :]
                                .rearrange("(a p) d -> p a d", p=128),
                            )
                            w2tiles[bg + 1] = w2t

                    for g4 in range(8):
                        if g4 < 2:
                            w1t = w1pre[g4]
                        else:
                            w1t = w1p.tile([128, KC, 512], bf16, tag="w1t",
                                           name="w1t")
                            (nc.sync if g4 % 2 == 0 else nc.scalar).dma_start(
                                w1t[:],
                                w1_d.ap()[:, g4 * 512:(g4 + 1) * 512]
                                .rearrange("(kc p) m -> p kc m", p=128),
                            )
                        w1tiles[g4] = w1t
                        for a in range(4):
                            mh = 4 * g4 + a
                            psa = ps_f1a.tile([128, 512], f32, tag="psf1a",
                                              name="psf1a")
                            for dc in range(KC):
                                nc.tensor.matmul(
                                    psa[:, 0:384],
                                    w1t[:, dc, a * 128:(a + 1) * 128],
                                    x1T[:, dc, 0:384],
                                    start=(dc == 0), stop=(dc == KC - 1),
                                )
                            nc.vector.tensor_scalar(
                                out=hT[:, mh, 0:384], in0=psa[:, 0:384],
                                scalar1=b1c[:, mh:mh + 1], scalar2=0.0,
                                op0=Alu.add, op1=Alu.max,
                            )
                        if g4 == 3:
                            x1_ln(3, xs[:, 3, :], g1bc, h1bc)
                            x1_transpose(3)
                        if g4 >= 3:
                            bstep(g4 - 3)
                    for bg in range(5, 8):
                        bstep(bg)
                    f2_finish(0, psyA[0])
                    f2_finish(1, psyA[1])

                # ---- FFN2 second half (token groups 2-3) + residual + LN2
                with tc.tile_pool(name="ps_f2b", bufs=1, space="PSUM") as ps_f2b:
                    psyB = [
                        [
                            ps_f2b.tile([128, 512], f32, tag=f"pyB{mt}{ncc}",
                                        name=f"pyB{mt}{ncc}")
                            for ncc in range(2)
                        ]
                        for mt in range(2)
                    ]
                    for wc in range(8):
                        w2t = w2p.tile([128, 4, D], bf16, tag="w2t", name="w2t")
                        (nc.sync if wc % 2 == 0 else nc.scalar).dma_start(
                            w2t[:],
                            w2_d.ap()[wc * 512:(wc + 1) * 512, :]
                            .rearrange("(a p) d -> p a d", p=128),
                        )
                        for a in range(4):
                            mh = 4 * wc + a
                            for mt in (2, 3):
                                for ncc in range(2):
                                    nc.tensor.matmul(
                                        psyB[mt - 2][ncc][:],
                                        hT[:, mh, mt * 128:(mt + 1) * 128],
                                        w2t[:, a, ncc * 512:(ncc + 1) * 512],
                                        start=(mh == 0), stop=False,
                                    )
                    f2_finish(2, psyB[0])
                    f2_finish(3, psyB[1])
    nc.compile()
    return nc


def _core_rows(c):
    # 3 reduce-scatter chunks: tokens 0-2047 scattered in 256-row blocks,
    # tokens 2048-3071 and 3072-4095 in 128-row blocks.
    return np.concatenate([
        c * 256 + np.arange(256),
        2048 + c * 128 + np.arange(128),
        3072 + c * 128 + np.arange(128),
    ])


def _in_maps(x, Wq, Wk, Wv, Wo, ln1_g, ln1_b, W1, b1, W2, b2, ln2_g, ln2_b):
    import ml_dtypes

    bf16 = ml_dtypes.bfloat16
    xf = np.ascontiguousarray(np.asarray(x, np.float32).reshape(N, D))
    xT = np.ascontiguousarray(xf.T.astype(bf16))
    Wq = np.asarray(Wq, np.float32)
    Wk = np.asarray(Wk, np.float32)
    Wv = np.asarray(Wv, np.float32)
    Wo = np.asarray(Wo, np.float32)
    bcast = lambda v: np.ascontiguousarray(
        np.broadcast_to(np.asarray(v, np.float32), (128, D))
    )
    common = {
        "xT": xT,
        "w1": np.ascontiguousarray(np.asarray(W1, np.float32).astype(bf16)),
        "w2": np.ascontiguousarray(np.asarray(W2, np.float32).astype(bf16)),
        "b1c": np.ascontiguousarray(np.asarray(b1, np.float32).reshape(MH, 128).T),
        "b2r": np.ascontiguousarray(np.asarray(b2, np.float32).reshape(1, D)),
        "g1bc": bcast(ln1_g), "h1bc": bcast(ln1_b),
        "g2bc": bcast(ln2_g), "h2bc": bcast(ln2_b),
        "ident": np.eye(128, dtype=np.float32),
        "identb": np.eye(128, dtype=np.float32).astype(bf16),
        "ones64": np.ones((1, 64), np.float32),
        "ones128": np.ones((1, 128), np.float32),
        "onesv": np.ones((128, 64), np.float32),
    }
    in_maps = []
    for c in range(8):
        h0 = 2 * c
        m = dict(common)
        m["wq"] = np.ascontiguousarray(
            Wq[h0:h0 + 2].transpose(1, 0, 2).reshape(D, 128).astype(bf16)
        )
        m["wk"] = np.ascontiguousarray(
            Wk[h0:h0 + 2].transpose(1, 0, 2).reshape(D, 128).astype(bf16)
        )
        m["wv"] = np.ascontiguousarray(
            Wv[h0:h0 + 2].transpose(1, 0, 2).reshape(D, 128).astype(bf16)
        )
        m["wo"] = np.ascontiguousarray(Wo[h0 * 64:h0 * 64 + 128, :])
        m["xs"] = np.ascontiguousarray(xf[_core_rows(c)])
        in_maps.append(m)
    return in_maps


def kernel(x, Wq, Wk, Wv, Wo, ln1_g, ln1_b, W1, b1, W2, b2, ln2_g, ln2_b):
    from concourse.bass_utils import run_bass_kernel_spmd

    if "nc" not in _CACHE:
        _CACHE["nc"] = _build()
    nc = _CACHE["nc"]
    in_maps = _in_maps(x, Wq, Wk, Wv, Wo, ln1_g, ln1_b, W1, b1, W2, b2, ln2_g, ln2_b)
    res = run_bass_kernel_spmd(nc, in_maps, core_ids=list(range(8)))
    out = np.empty((N, D), np.float32)
    for c in range(8):
        out[_core_rows(c)] = res.results[c]["y"]
    return out.reshape(B, S, D)
